# revision 7
# baseline (speedup 1.0000x reference)
"""Bass/Trainium2 kernel for nn_HadamardClassifier.

Math: out = -scale * l2norm(x) @ H + bias, with H = H_16384[:2048, :14951]
(Sylvester). Since H_16384 = H_8 (x) H_2048 and rows < 2048 hit only row 0 of
the H_8 factor (all +1), H is just H_2048 tiled horizontally:
    out[:, j] = (x * (-scale/||x||_2)) @ H_2048[:, j % 2048] + bias[j]

Sharding: batch-parallel across 8 cores (512 rows each).

Numerics: H_2048 entries are exactly +-1 so it is cast to bf16 losslessly and
the matmul runs on the PE at bf16 rate (2x fp32). x is split into bf16 hi+lo
parts (x ~= xh + xl) and both pass through the same accumulation group, so
Z = xh@H + xl@H matches the f32 product to ~1e-6 relative.
"""

import math

import numpy as np

B, IN, OUT = 4096, 2048, 14951
NCORES = 8
BLOC = B // NCORES  # 512
P = 128
PERIOD = 2048
NFULL = 7  # full 2048-wide output blocks
TAIL = OUT - NFULL * PERIOD  # 615
EPS = 1e-12
NCB = BLOC // P  # 4 batch chunks per core
NIC = IN // P  # 16 contraction chunks
NJS = PERIOD // 512  # 4 column slabs of 512
# bias packed js-major: per js, its 512-col piece of every block, 512-padded
NBLK_JS = [8, 8, 7, 7]  # js1's 8th seg is the 103-wide tail (padded)
OFF_JS = [0, 4096, 8192, 11776]
BIAS_PACK = 15360

_CACHE = {}
LAST_RESULT = None
PROFILE = False


def _build(scale_val: float):
    from contextlib import ExitStack

    import concourse.bass as bass
    import concourse.mybir as mybir
    import concourse.tile as tile
    from concourse import bacc, masks

    f32 = mybir.dt.float32
    bf16 = mybir.dt.bfloat16
    nc = bacc.Bacc("TRN2", target_bir_lowering=False, debug=False,
                   num_devices=NCORES)

    x_d = nc.dram_tensor("x", [BLOC, IN], f32, kind="ExternalInput")
    h_d = nc.dram_tensor("h", [IN, PERIOD], bf16, kind="ExternalInput")
    b_d = nc.dram_tensor("bias", [1, BIAS_PACK], f32, kind="ExternalInput")
    o_d = nc.dram_tensor("out", [BLOC, OUT], f32, kind="ExternalOutput")

    # [2048 rows] -> [p, ic] view so each SBUF partition p holds rows ic*128+p
    h_v = h_d[:, :].rearrange("(ic p) j -> p ic j", p=P)
    # main output region as [rows, blk, col-in-block]
    o_main = o_d[:, 0 : NFULL * PERIOD].rearrange("r (blk c) -> r blk c",
                                                  c=PERIOD)

    with tile.TileContext(nc) as tc, ExitStack() as ctx:
        p_const = ctx.enter_context(tc.tile_pool(name="const", bufs=1))
        p_x = ctx.enter_context(tc.tile_pool(name="xload", bufs=2))
        p_w = ctx.enter_context(tc.tile_pool(name="work", bufs=1))
        p_ss = ctx.enter_context(tc.tile_pool(name="small", bufs=8))
        p_xth = ctx.enter_context(tc.tile_pool(name="xth", bufs=NCB))
        p_xtl = ctx.enter_context(tc.tile_pool(name="xtl", bufs=NCB))
        p_h = ctx.enter_context(tc.tile_pool(name="hslab", bufs=6))
        p_z = ctx.enter_context(tc.tile_pool(name="zsb", bufs=4))
        p_o = ctx.enter_context(tc.tile_pool(name="ostage", bufs=4))
        p_pst = ctx.enter_context(
            tc.tile_pool(name="psum_t", bufs=2, space="PSUM"))
        p_psz = ctx.enter_context(
            tc.tile_pool(name="psum_z", bufs=6, space="PSUM"))

        ident = p_const.tile([P, P], f32, tag="ident")
        masks.make_identity(nc, ident[:])

        # HAM warmup: keep the PE busy from t=0 so the clock gate opens
        # (4/8 -> 8/8) before the real matmul stream starts; otherwise the
        # first ~30us of matmuls run at 1.2 GHz
        warm = p_pst.tile([P, P], f32, tag="pst")
        for _ in range(16):
            nc.tensor.matmul(warm[:], ident[:], ident[:], start=True,
                             stop=True)

        # bias: load packed row into partition 0; broadcast in per-block
        # chunks on gpsimd, interleaved into phase 2 so the SWDGE out-DMA
        # queue is never blocked long
        bias_rep = p_const.tile([P, BIAS_PACK], f32, tag="bias_rep")
        nc.sync.dma_start(out=bias_rep[0:1, :], in_=b_d[:, :])

        def bcast_js(js):
            o0 = OFF_JS[js]
            for blk in range(NBLK_JS[js]):
                a = o0 + blk * 512
                nc.gpsimd.partition_broadcast(bias_rep[:, a : a + 512],
                                              bias_rep[0:1, a : a + 512])

        bcast_js(0)
        # remaining 22 chunks dribbled out 2 per iteration (deadline: js's
        # first add), so the SWDGE out-DMA queue never blocks long
        bcast_rest = [(js, blk) for js in range(1, NJS)
                      for blk in range(NBLK_JS[js])]

        # ---- phase 1: load x, l2-normalize rows (folding -scale),
        # transpose, split into bf16 hi+lo
        xths, xtls = [], []
        hq_tiles = {}
        for cb in range(NCB):
            xnat = p_x.tile([P, IN], f32, tag="xnat")
            nc.sync.dma_start(out=xnat[:], in_=x_d[cb * P : (cb + 1) * P, :])
            # interleave js0's H halves into the load queue
            hq = p_h.tile([P, 8, 512], bf16, tag="hslab")
            half = cb % 2
            js = cb // 2
            nc.sync.dma_start(
                out=hq[:],
                in_=h_v[:, half * 8 : half * 8 + 8, js * 512 : js * 512 + 512])
            hq_tiles[(js, half)] = hq

            sq = p_w.tile([P, IN], bf16, tag="work")
            ss = p_ss.tile([P, 1], f32, tag="ss")
            nc.scalar.activation(sq[:], xnat[:],
                                 mybir.ActivationFunctionType.Square,
                                 accum_out=ss[:])
            nc.vector.tensor_scalar_max(ss[:], ss[:], EPS)
            nrm = p_ss.tile([P, 1], f32, tag="nrm")
            nc.scalar.sqrt(nrm[:], ss[:])
            inv = p_ss.tile([P, 1], f32, tag="inv")
            nc.vector.reciprocal(inv[:], nrm[:])
            mult = p_ss.tile([P, 1], f32, tag="mult")
            nc.vector.tensor_scalar_mul(mult[:], inv[:], -scale_val)

            # scale rows in place on ACT (DVE is the busier engine)
            nc.scalar.mul(xnat[:], xnat[:], mult[:, 0:1])

            xth = p_xth.tile([P, NIC, P], bf16, tag="xth")
            xtl = p_xtl.tile([P, NIC, P], bf16, tag="xtl")
            for ic in range(NIC):
                pst = p_pst.tile([P, P], f32, tag="pst")
                nc.tensor.transpose(pst[:], xnat[:, ic * P : (ic + 1) * P],
                                    ident[:])
                # hi = bf16(xn^T); lo = bf16(xn^T - hi)
                nc.scalar.copy(xth[:, ic, :], pst[:])
                nc.vector.tensor_sub(xtl[:, ic, :], pst[:], xth[:, ic, :])
            xths.append(xth)
            xtls.append(xtl)

        # ---- phase 2: Z = xn' @ H_2048 slab by slab; add bias; store
        for js in range(NJS):
            for half in range(2):
                if (js, half) not in hq_tiles:
                    hq = p_h.tile([P, 8, 512], bf16, tag="hslab")
                    nc.sync.dma_start(
                        out=hq[:],
                        in_=h_v[:, half * 8 : half * 8 + 8,
                                js * 512 : js * 512 + 512])
                    hq_tiles[(js, half)] = hq

        for js in range(NJS):
            c0 = js * 512
            boff = OFF_JS[js]
            for cb in range(NCB):
                last = (js == NJS - 1 and cb == NCB - 1)
                # the very last iteration runs in two 256-col halves so the
                # post-matmul adds+store tail is half as long
                col_chunks = [(0, 256), (256, 256)] if last else [(0, 512)]
                psz = p_psz.tile([P, 512], f32, tag="psz")
                for ic in range(NIC):
                    hap = hq_tiles[(js, ic // 8)][:, ic % 8, :]
                    nc.tensor.matmul(psz[:], xths[cb][:, ic, :], hap,
                                     start=(ic == 0), stop=False)
                    nc.tensor.matmul(psz[:], xtls[cb][:, ic, :], hap,
                                     start=False, stop=(ic == NIC - 1))
                for w0, wn in col_chunks:
                    zsb = p_z.tile([P, 512], f32, tag="zsb")
                    nc.scalar.copy(zsb[:, 0:wn], psz[:, w0 : w0 + wn])

                    ost1 = p_o.tile([P, 4, 512], f32, tag="ostage")
                    ost2 = p_o.tile([P, 4, 512], f32, tag="ostage")
                    for blk in range(4):
                        nc.vector.tensor_add(
                            ost1[:, blk, 0:wn], zsb[:, 0:wn],
                            bias_rep[:, boff + blk * 512 + w0 :
                                     boff + blk * 512 + w0 + wn])
                    for blk in range(4, 7):
                        nc.vector.tensor_add(
                            ost2[:, blk - 4, 0:wn], zsb[:, 0:wn],
                            bias_rep[:, boff + blk * 512 + w0 :
                                     boff + blk * 512 + w0 + wn])
                    tw = 512 if js == 0 else (103 if js == 1 else 0)
                    tw = max(min(tw - w0, wn), 0)
                    if tw:
                        nc.vector.tensor_add(
                            ost2[:, 3, 0:tw], zsb[:, 0:tw],
                            bias_rep[:, boff + 7 * 512 + w0 :
                                     boff + 7 * 512 + w0 + tw])

                    r0 = cb * P
                    nc.gpsimd.dma_start(
                        out=o_main[r0 : r0 + P, 0:4,
                                   c0 + w0 : c0 + w0 + wn],
                        in_=ost1[:, :, 0:wn])
                    nc.gpsimd.dma_start(
                        out=o_main[r0 : r0 + P, 4:7,
                                   c0 + w0 : c0 + w0 + wn],
                        in_=ost2[:, 0:3, 0:wn])
                    if tw:
                        nc.gpsimd.dma_start(
                            out=o_d[r0 : r0 + P, NFULL * PERIOD + c0 + w0 :
                                    NFULL * PERIOD + c0 + w0 + tw],
                            in_=ost2[:, 3, 0:tw])
                # stagger the remaining bias broadcasts behind out-DMAs
                for _ in range(3):
                    if bcast_rest:
                        bjs, bblk = bcast_rest.pop(0)
                        a = OFF_JS[bjs] + bblk * 512
                        nc.gpsimd.partition_broadcast(
                            bias_rep[:, a : a + 512], bias_rep[0:1, a : a + 512])

    nc.compile()
    return nc


def _pack_bias(bias: np.ndarray) -> np.ndarray:
    pack = np.zeros((1, BIAS_PACK), dtype=np.float32)
    for js in range(NJS):
        for blk in range(NBLK_JS[js]):
            src0 = blk * PERIOD + js * 512
            seg = bias[src0 : src0 + 512]
            pack[0, OFF_JS[js] + blk * 512 : OFF_JS[js] + blk * 512 + len(seg)] = seg
    return pack


def kernel(x, hadamard, scale, bias):
    global LAST_RESULT
    import ml_dtypes
    from concourse.bass_utils import run_bass_kernel_spmd

    x = np.ascontiguousarray(np.asarray(x, dtype=np.float32))
    hadamard = np.asarray(hadamard, dtype=np.float32)
    bias = np.asarray(bias, dtype=np.float32)
    scale_val = float(np.asarray(scale).reshape(-1)[0])

    h2 = np.ascontiguousarray(hadamard[:, :PERIOD])
    # the whole kernel rests on the 2048-periodicity of the weight columns
    for k in range(1, NFULL):
        assert np.array_equal(hadamard[:, k * PERIOD : (k + 1) * PERIOD], h2), (
            "hadamard is not 2048-periodic; kernel assumption violated")
    assert np.array_equal(hadamard[:, NFULL * PERIOD :], h2[:, :TAIL])
    h2b = h2.astype(ml_dtypes.bfloat16)
    assert np.array_equal(h2b.astype(np.float32), h2), "H not bf16-exact"

    key = scale_val
    if key not in _CACHE:
        _CACHE[key] = _build(scale_val)
    nc = _CACHE[key]

    bias_pack = _pack_bias(bias)
    in_maps = [
        {"x": np.ascontiguousarray(x[c * BLOC : (c + 1) * BLOC]),
         "h": h2b, "bias": bias_pack}
        for c in range(NCORES)
    ]
    res = run_bass_kernel_spmd(nc, in_maps, list(range(NCORES)),
                               trace=PROFILE)
    LAST_RESULT = res
    out = np.concatenate([res.results[c]["out"] for c in range(NCORES)],
                         axis=0)
    return out
